# revision 2
# baseline (speedup 1.0000x reference)
"""Trainium2 Bass kernel for AttentiveRelationalModuleUniformObs (v5).

Math (per sample b over N=256 neighbors, D=64, LAT=128, EC=32):
    feat   = relu(nbr @ Wf + bf)            [N, LAT]
    enc    = relu(nbr @ Wc + bc)            [N, EC]
    att    = softmax_N(enc @ Wa2)           [N, LAT]   (shift-invariance:
             self/mean/ba logit terms are constant over N and cancel)
    out[b] = relu((att * feat).sum(N) @ Wl + bl)

Strategy:
  - All matmuls bf16 (full PE rate, halved HBM traffic); layout A
    (LAT on partitions, neighbors on the free axis).
  - ONE custom DVE op per sample (MUL_RELUBIAS_SCANSEL) computes both
    softmax reductions in a single 1x pass over (e, F):
        body   = select(Idx >= 255, scan(+, e), e * relu(F + bf))
        out    = prod prefixes, except out[:, 255] = sum_n e  (= den)
        accum  = sum(body) = (num - prod[255]) + den
    A tiny Pool op recovers prod[255]; the finale solves
        num = accum - den + prod255.
    So DVE does 4x[128,256] ops per group and nothing else.
  - ACT: one exp per group [128,1024] PSUM->SBUF bf16.
  - Pool: enc-relu (tensor_scalar add-bias/max-0) + the prod255 taps.
  - Engine budget per 4-sample group ~ DVE 1.57us / Pool 1.38us /
    ACT 1.04us / PE 1.30us / DMA 0.78us -> DVE-paced ~51us/core.
"""

import numpy as np

B, N, D, LAT, EC = 1024, 256, 64, 128, 32
M = 8           # cores
S = B // M      # samples per core (128)
G = S // 4      # main-loop iterations per core (4 samples each)

_CACHE = {}

_MRS_NAME = "MUL_RELUBIAS_SCANSEL"
_MRS_ROW = 17  # custom-DVE opcode rows 1..16 are taken by production ops


def _register_mrs():
    """Register the custom DVE op at runtime (pre-seeding the compile
    cache in place of a pinned uops sha):
        body  = select(Idx >= s0, scan(add, in0), in0 * relu(in1 + s1))
        accum = sum(body)
    """
    import concourse.dve_ops as dops
    from concourse.dve_spec import (
        AluOp, Spec, Src0, Src1, C0, C1, Zero, Idx, relu, scan, select, lower,
    )
    from concourse.dve_spec import _has_src1
    from concourse.dve_uop import DveOpSpec
    from operator import add

    if _MRS_NAME in dops._SUB_OPCODE_FOR_NAME:
        return next(op for op in dops.OPS if op.name == _MRS_NAME)

    def _ref(in0, in1, c0, c1, c2):
        in0f = in0.astype(np.float32).reshape(in0.shape[0], -1)
        in1f = in1.astype(np.float32).reshape(in0.shape[0], -1)
        prod = in0f * np.maximum(np.nan_to_num(in1f), 0.0)
        cump = np.cumsum(prod, axis=-1)
        idx = np.arange(in0f.shape[-1])[None, :]
        body = np.where(idx >= c0, cump, in0f).astype(np.float32)
        return (
            body.reshape(in0.shape),
            body.sum(axis=-1, keepdims=True),
        )

    body = select(Idx >= C0, scan(AluOp.ADD, Src0 * relu(Src1)), Src0)
    spec = Spec(body=body, accum=add, accum_init=Zero, reference=_ref)
    op = dops.DveOp(_MRS_NAME, spec, subdim=False, uops_sha={})
    dops.OPS.append(op)
    dops.CUSTOM_DVE_SPECS[_MRS_NAME] = spec
    dops._SUB_OPCODE_FOR_NAME[_MRS_NAME] = _MRS_ROW
    for ver in ("v3", "v4"):
        compiled = DveOpSpec(
            name=_MRS_NAME,
            opcode=_MRS_ROW,
            uops=lower(spec, ver=ver),
            rd1_en=_has_src1(spec),
        )
        dops._COMPILE_CACHE[(_MRS_NAME, ver)] = compiled
    return op


def _build_bass(with_bias: bool):
    import concourse.bacc as bacc
    import concourse.tile as tile
    from concourse import mybir

    mrs_op = _register_mrs()

    f32 = mybir.dt.float32
    bf16 = mybir.dt.bfloat16
    AF = mybir.ActivationFunctionType
    ALU = mybir.AluOpType

    nc = bacc.Bacc("TRN2", target_bir_lowering=False)

    # DRAM I/O.  nbrT: [S*D, N] bf16, host pre-transposed (D on partitions).
    nbrT_d = nc.dram_tensor("nbrT", [S * D, N], bf16, kind="ExternalInput")
    # bf16 weights packed into one tensor (one DMA):
    # cols [0:128) WfH0 | [128:256) WfH1 | [256:320) WcS2 blockdiag
    #      | [320:448) WaH0 (rows 0:32) | [448:576) WaH1 (rows 32:64)
    #      | [576:704) Wl | [704:832) bl (row 0) | [832:960) bf (row 0)
    #      | [960:964) f32 bias cols (bitcast): bf | bc-tiled-2x
    wtsb_d = nc.dram_tensor("wtsb", [128, 964], bf16, kind="ExternalInput")
    out_d = nc.dram_tensor("out", [S, LAT], f32, kind="ExternalOutput")

    with tile.TileContext(nc) as tc:
        from contextlib import ExitStack

        with ExitStack() as ctx:
            singles = ctx.enter_context(tc.tile_pool(name="singles", bufs=1))
            nbr_pool = ctx.enter_context(tc.tile_pool(name="nbr", bufs=8))
            enc_pool = ctx.enter_context(tc.tile_pool(name="enc", bufs=4))
            e_pool = ctx.enter_context(tc.tile_pool(name="e", bufs=5))
            f255_pool = ctx.enter_context(tc.tile_pool(name="f255", bufs=4))
            # PSUM: att 2x[128,4,256] (4 banks) + feat 3x[128,2,256] (3) +
            # comm 1x[64,2,256] (1) = 8 banks; the finale reuses the feat tag.
            ps_att = ctx.enter_context(
                tc.tile_pool(name="ps_att", bufs=2, space="PSUM")
            )
            ps_feat = ctx.enter_context(
                tc.tile_pool(name="ps_feat", bufs=3, space="PSUM")
            )
            ps_comm = ctx.enter_context(
                tc.tile_pool(name="ps_comm", bufs=1, space="PSUM")
            )

            wtsb = singles.tile([128, 964], bf16)
            nc.sync.dma_start(out=wtsb, in_=wtsb_d[:, :])
            wtsf = wtsb[:, 960:964].bitcast(f32)
            wfh = [wtsb[:, 0:128], wtsb[:, 128:256]]
            wc_sb = wtsb[:, 256:320]
            wah = [wtsb[0:64, 320:448], wtsb[0:64, 448:576]]
            wl_sb = wtsb[:, 576:704]
            blr = wtsb[0:1, 704:832]
            bf_row = wtsb[0:1, 832:960]
            bc_col = wtsf[0:64, 1:2]
            ones1 = singles.tile([1, LAT], bf16)
            nc.vector.memset(ones1, 1.0)
            ones512 = singles.tile([1, 2, N], bf16)
            nc.vector.memset(ones512, 1.0)
            # warm the Exp activation table while the weight DMAs are in
            # flight so LoadActFuncSet isn't on the first group's chain
            warm = singles.tile([1, 1], f32)
            nc.vector.memset(warm, 0.0)
            warm2 = singles.tile([1, 1], f32)
            nc.scalar.activation(out=warm2, in_=warm, func=AF.Exp)

            # custom-op outputs: prod-prefix scratch (col 255 = den),
            # accum column (= num - prod255 + den), prod255 taps
            scr_all = singles.tile([128, G, 4, N], bf16)
            acc_blk = singles.tile([LAT, S], f32)
            p255_blk = singles.tile([LAT, S], f32)

            nbrT_ap = nbrT_d[:, :]

            # Software-pipelined: comm+enc for group g+1 are emitted before
            # the back half of group g so Pool's enc-relu stays off the
            # enc->att->exp critical cycle.
            nb_live = {}
            enc_live = {}

            def emit_front(g):
                # 4 samples: partition p = 64*h + d, free index t; sample
                # s = 4g + 2t + h.  1KB-contiguous rows in DRAM per (t,p).
                r0 = 4 * g * D
                nbg = nbr_pool.tile([128, 2, N], bf16, tag="nbr")
                nc.sync.dma_start(
                    out=nbg,
                    in_=nbrT_ap[r0 : r0 + 256, :].rearrange(
                        "(t p) n -> p t n", p=128
                    ),
                )
                nb_live[g] = nbg

                # communication encoder via block-diagonal WcS2:
                # C[32h+c, t, n] = enc_c(sample 2t+h)
                C = ps_comm.tile([64, 2, N], f32, tag="C")
                nc.tensor.matmul(
                    out=C, lhsT=wc_sb, rhs=nbg, start=True, stop=True
                )
                enc = enc_pool.tile([64, 2, N], bf16, tag="enc")
                nc.scalar.activation(
                    out=enc, in_=C, func=AF.Relu, bias=bc_col, scale=1.0
                )
                enc_live[g] = enc

            emit_front(0)
            for g in range(G):
                if g + 1 < G:
                    emit_front(g + 1)
                nb = nb_live.pop(g)
                enc = enc_live.pop(g)

                # attention logits first (exp gates the DVE chain):
                # A[k, (h,t), n] = logit_k(sample 2t+h); each MM-h writes
                # one contiguous PSUM bank of the A tile
                A = ps_att.tile([128, 2, 2, N], f32, tag="A")
                for h in range(2):
                    nc.tensor.matmul(
                        out=A[:, h, :, :], lhsT=wah[h], rhs=enc[:, :, :],
                        start=True, stop=True,
                    )

                # exp of the whole group in one ACT op, PSUM -> SBUF bf16
                e = e_pool.tile([128, 2, 2, N], bf16, tag="e")
                nc.scalar.activation(out=e, in_=A, func=AF.Exp)

                # feature encoder (transposed): F_h[k, t, n] = feat_k(2t+h);
                # when bf != 0 it is added via an accumulating K=1 matmul
                F = []
                for h in range(2):
                    Fh = ps_feat.tile([128, 2, N], f32, tag="F")
                    nc.tensor.matmul(
                        out=Fh, lhsT=wfh[h], rhs=nb[:, :, :],
                        start=True, stop=not with_bias,
                    )
                    if with_bias:
                        nc.tensor.matmul(
                            out=Fh, lhsT=bf_row, rhs=ones512[:, :, :],
                            start=False, stop=True,
                        )
                    F.append(Fh)

                # fused num/den: one custom DVE op per sample
                for h in range(2):
                    for t in range(2):
                        j = 2 * t + h
                        s = 4 * g + j
                        nc.vector._custom_dve(
                            mrs_op,
                            out=scr_all[:, g, j, :],
                            in0=e[:, h, t, :],
                            in1=F[h][:, t, :],
                            s0=float(N - 1),
                            accum_out=acc_blk[:, s : s + 1],
                        )

                # e255 tap on Pool (SBUF->SBUF copy, sample order s=2t+h):
                # the finale solves den = accum - num + e255
                e4 = p255_blk[:, 4 * g : 4 * g + 4].rearrange(
                    "p (t h2) -> p h2 t", h2=2
                )
                nc.gpsimd.tensor_scalar(
                    out=e4, in0=e[:, :, :, N - 1], scalar1=1.0, scalar2=None,
                    op0=ALU.mult,
                )

            # finale: num = scr[..., 255]; den = acc - num + e255;
            # out = relu((num/den) @ Wl + bl).  Two half-pipelines so the
            # first half's matmul/relu/DMA overlaps the second's compute.
            num_v = scr_all[:, :, :, N - 1].rearrange("p g j -> p (g j)")
            den_t = singles.tile([LAT, S], f32)
            rden = singles.tile([LAT, S], f32)
            aggT = singles.tile([LAT, S], bf16)
            out_sb = singles.tile([S, LAT], f32)
            H = S // 2
            for hf in range(2):
                sl = slice(H * hf, H * hf + H)
                nc.vector.tensor_tensor(
                    out=den_t[:, sl], in0=acc_blk[:, sl], in1=num_v[:, sl],
                    op=ALU.subtract,
                )
                nc.vector.tensor_add(
                    out=den_t[:, sl], in0=den_t[:, sl], in1=p255_blk[:, sl]
                )
                nc.vector.reciprocal(out=rden[:, sl], in_=den_t[:, sl])
                nc.vector.tensor_mul(
                    out=aggT[:, sl], in0=num_v[:, sl], in1=rden[:, sl]
                )
                out_ps_w = ps_feat.tile([128, 2, N], f32, tag="F")
                out_ps = out_ps_w.rearrange("p t n -> p (t n)")[0:H, 0:LAT]
                nc.tensor.matmul(
                    out=out_ps, lhsT=aggT[:, sl], rhs=wl_sb,
                    start=True, stop=False,
                )
                nc.tensor.matmul(
                    out=out_ps, lhsT=ones1[:, 0:H], rhs=blr,
                    start=False, stop=True,
                )
                nc.scalar.activation(
                    out=out_sb[sl, :], in_=out_ps, func=AF.Relu
                )
                nc.sync.dma_start(out=out_d[sl, :], in_=out_sb[sl, :])

    nc.finalize()
    return nc


def _host_prep(inputs):
    import ml_dtypes

    bf = np.asarray(inputs["bf"], dtype=np.float32)
    bc = np.asarray(inputs["bc"], dtype=np.float32)
    bl = np.asarray(inputs["bl"], dtype=np.float32)
    Wf = np.asarray(inputs["Wf"], dtype=np.float32)
    Wc = np.asarray(inputs["Wc"], dtype=np.float32)
    Wa = np.asarray(inputs["Wa"], dtype=np.float32)
    Wl = np.asarray(inputs["Wl"], dtype=np.float32)
    Wa2 = Wa[EC : 2 * EC]  # only the enc_comm block survives softmax shift

    nbr = np.asarray(inputs["neighbor_data"], dtype=np.float32)
    # [M, S, N, D] -> [M, S, D, N] -> [M, S*D, N], bf16
    nbrT = (
        np.ascontiguousarray(nbr.reshape(M, S, N, D).transpose(0, 1, 3, 2))
        .reshape(M, S * D, N)
        .astype(ml_dtypes.bfloat16)
    )

    wtsb = np.zeros((128, 964), dtype=ml_dtypes.bfloat16)
    wtsb[0:64, 0:128] = Wf
    wtsb[64:128, 128:256] = Wf
    wtsb[0:64, 256:288] = Wc
    wtsb[64:128, 288:320] = Wc
    wtsb[0:32, 320:448] = Wa2
    wtsb[32:64, 448:576] = Wa2
    wtsb[:, 576:704] = Wl
    wtsb[0, 704:832] = bl
    wtsb[0, 832:960] = bf

    wtsf = np.zeros((128, 2), dtype=np.float32)
    wtsf[:, 0] = bf
    wtsf[0:64, 1] = np.tile(bc, 2)
    wtsb[:, 960:964] = wtsf.view(np.uint16).view(ml_dtypes.bfloat16)

    return [{"nbrT": nbrT[c], "wtsb": wtsb} for c in range(M)]


def kernel(**inputs) -> np.ndarray:
    from concourse.bass_utils import run_bass_kernel_spmd

    with_bias = bool(np.any(np.asarray(inputs["bf"])))
    key = ("nc", with_bias)
    if key not in _CACHE:
        _CACHE[key] = _build_bass(with_bias)
    nc = _CACHE[key]

    in_maps = _host_prep(inputs)
    res = run_bass_kernel_spmd(nc, in_maps, list(range(M)))
    out = np.concatenate(
        [np.asarray(res.results[c]["out"]) for c in range(M)], axis=0
    )
    return out.astype(np.float32)


# revision 4
# speedup vs baseline: 7567.1833x; 7567.1833x over previous
"""Trainium2 Bass kernel for AttentiveRelationalModuleUniformObs (v5).

Math (per sample b over N=256 neighbors, D=64, LAT=128, EC=32):
    feat   = relu(nbr @ Wf + bf)            [N, LAT]
    enc    = relu(nbr @ Wc + bc)            [N, EC]
    att    = softmax_N(enc @ Wa2)           [N, LAT]   (shift-invariance:
             self/mean/ba logit terms are constant over N and cancel)
    out[b] = relu((att * feat).sum(N) @ Wl + bl)

Strategy:
  - All matmuls bf16 (full PE rate, halved HBM traffic); layout A
    (LAT on partitions, neighbors on the free axis).
  - ONE custom DVE op per sample (MUL_RELUBIAS_SCANSEL) computes both
    softmax reductions in a single 1x pass over (e, F):
        body   = select(Idx >= 255, scan(+, e), e * relu(F + bf))
        out    = prod prefixes, except out[:, 255] = sum_n e  (= den)
        accum  = sum(body) = (num - prod[255]) + den
    A tiny Pool op recovers prod[255]; the finale solves
        num = accum - den + prod255.
    So DVE does 4x[128,256] ops per group and nothing else.
  - ACT: one exp per group [128,1024] PSUM->SBUF bf16.
  - Pool: enc-relu (tensor_scalar add-bias/max-0) + the prod255 taps.
  - Engine budget per 4-sample group ~ DVE 1.57us / Pool 1.38us /
    ACT 1.04us / PE 1.30us / DMA 0.78us -> DVE-paced ~51us/core.
"""

import numpy as np

B, N, D, LAT, EC = 1024, 256, 64, 128, 32
M = 8           # cores
S = B // M      # samples per core (128)
G = S // 4      # main-loop iterations per core (4 samples each)

_CACHE = {}

_MRS_NAME = "MUL_RELUBIAS_SCANSEL"
_MRS_ROW = 17  # custom-DVE opcode rows 1..16 are taken by production ops


def _register_mrs():
    """Register the custom DVE op at runtime (pre-seeding the compile
    cache in place of a pinned uops sha):
        body  = select(Idx >= s0, scan(add, in0), in0 * relu(in1 + s1))
        accum = sum(body)
    """
    import concourse.dve_ops as dops
    from concourse.dve_spec import (
        AluOp, Spec, Src0, Src1, C0, C1, Zero, Idx, relu, scan, select, lower,
    )
    from concourse.dve_spec import _has_src1
    from concourse.dve_uop import DveOpSpec
    from operator import add

    if _MRS_NAME in dops._SUB_OPCODE_FOR_NAME:
        return next(op for op in dops.OPS if op.name == _MRS_NAME)

    def _ref(in0, in1, c0, c1, c2):
        in0f = in0.astype(np.float32).reshape(in0.shape[0], -1)
        in1f = in1.astype(np.float32).reshape(in0.shape[0], -1)
        prod = in0f * np.maximum(np.nan_to_num(in1f), 0.0)
        cump = np.cumsum(prod, axis=-1)
        idx = np.arange(in0f.shape[-1])[None, :]
        body = np.where(idx >= c0, cump, in0f).astype(np.float32)
        return (
            body.reshape(in0.shape),
            body.sum(axis=-1, keepdims=True),
        )

    body = select(Idx >= C0, scan(AluOp.ADD, Src0 * relu(Src1)), Src0)
    spec = Spec(body=body, accum=add, accum_init=Zero, reference=_ref)
    op = dops.DveOp(_MRS_NAME, spec, subdim=False, uops_sha={})
    dops.OPS.append(op)
    dops.CUSTOM_DVE_SPECS[_MRS_NAME] = spec
    dops._SUB_OPCODE_FOR_NAME[_MRS_NAME] = _MRS_ROW
    for ver in ("v3", "v4"):
        compiled = DveOpSpec(
            name=_MRS_NAME,
            opcode=_MRS_ROW,
            uops=lower(spec, ver=ver),
            rd1_en=_has_src1(spec),
        )
        dops._COMPILE_CACHE[(_MRS_NAME, ver)] = compiled
    return op


def _build_bass(with_bias: bool, bench_reps: int = 1):
    import concourse.bacc as bacc
    import concourse.tile as tile
    from concourse import mybir

    mrs_op = _register_mrs()

    f32 = mybir.dt.float32
    bf16 = mybir.dt.bfloat16
    AF = mybir.ActivationFunctionType
    ALU = mybir.AluOpType

    nc = bacc.Bacc("TRN2", target_bir_lowering=False)

    # DRAM I/O.  nbrT: [S*D, N] bf16, host pre-transposed (D on partitions).
    nbrT_d = nc.dram_tensor("nbrT", [S * D, N], bf16, kind="ExternalInput")
    # bf16 weights packed into one tensor (one DMA):
    # cols [0:128) WfH0 | [128:256) WfH1 | [256:320) WcS2 blockdiag
    #      | [320:448) WaH0 (rows 0:32) | [448:576) WaH1 (rows 32:64)
    #      | [576:704) Wl | [704:832) bl (row 0) | [832:960) bf (row 0)
    #      | [960:964) f32 bias cols (bitcast): bf | bc-tiled-2x
    wtsb_d = nc.dram_tensor("wtsb", [128, 964], bf16, kind="ExternalInput")
    out_d = nc.dram_tensor("out", [S, LAT], f32, kind="ExternalOutput")

    with tile.TileContext(nc) as tc:
        from contextlib import ExitStack

        with ExitStack() as ctx:
            singles = ctx.enter_context(tc.tile_pool(name="singles", bufs=1))
            nbr_pool = ctx.enter_context(tc.tile_pool(name="nbr", bufs=5))
            enc_pool = ctx.enter_context(tc.tile_pool(name="enc", bufs=4))
            e_pool = ctx.enter_context(tc.tile_pool(name="e", bufs=3))
            f255_pool = ctx.enter_context(tc.tile_pool(name="f255", bufs=4))
            # PSUM: att 2x[128,4,256] (4 banks) + feat 3x[128,2,256] (3) +
            # comm 1x[64,2,256] (1) = 8 banks; the finale reuses the feat tag.
            ps_att = ctx.enter_context(
                tc.tile_pool(name="ps_att", bufs=2, space="PSUM")
            )
            ps_feat = ctx.enter_context(
                tc.tile_pool(name="ps_feat", bufs=3, space="PSUM")
            )
            ps_comm = ctx.enter_context(
                tc.tile_pool(name="ps_comm", bufs=1, space="PSUM")
            )

            wtsb = singles.tile([128, 964], bf16)
            nc.sync.dma_start(out=wtsb, in_=wtsb_d[:, :])
            wtsf = wtsb[:, 960:964].bitcast(f32)
            wfh = [wtsb[:, 0:128], wtsb[:, 128:256]]
            wc_sb = wtsb[:, 256:320]
            wah = [wtsb[0:64, 320:448], wtsb[0:64, 448:576]]
            wl_sb = wtsb[:, 576:704]
            blr = wtsb[0:1, 704:832]
            bf_row = wtsb[0:1, 832:960]
            bc_col = wtsf[0:64, 1:2]
            ones1 = singles.tile([1, LAT], bf16)
            nc.vector.memset(ones1, 1.0)
            ones512 = singles.tile([1, 2, N], bf16)
            nc.vector.memset(ones512, 1.0)
            # warm the Exp activation table while the weight DMAs are in
            # flight so LoadActFuncSet isn't on the first group's chain
            warm = singles.tile([1, 1], f32)
            nc.vector.memset(warm, 0.0)
            warm2 = singles.tile([1, 1], f32)
            nc.scalar.activation(out=warm2, in_=warm, func=AF.Exp)

            # custom-op outputs: prod-prefix scratch (col 255 = den),
            # accum column (= num - prod255 + den), prod255 taps
            scr_all = singles.tile([128, G, 4, N], bf16)
            acc_blk = singles.tile([LAT, S], f32)
            p255_blk = singles.tile([LAT, S], f32)

            nbrT_ap = nbrT_d[:, :]

            if bench_reps > 1:
                ctx.enter_context(tc.For_i(0, bench_reps))

            # Software-pipelined: comm+enc for group g+1 are emitted before
            # the back half of group g so Pool's enc-relu stays off the
            # enc->att->exp critical cycle.
            nb_live = {}
            enc_live = {}

            def emit_front(g):
                # 4 samples: partition p = 64*h + d, free index t; sample
                # s = 4g + 2t + h.  1KB-contiguous rows in DRAM per (t,p).
                r0 = 4 * g * D
                nbg = nbr_pool.tile([128, 2, N], bf16, tag="nbr")
                nc.sync.dma_start(
                    out=nbg,
                    in_=nbrT_ap[r0 : r0 + 256, :].rearrange(
                        "(t p) n -> p t n", p=128
                    ),
                )
                nb_live[g] = nbg

                # communication encoder via block-diagonal WcS2:
                # C[32h+c, t, n] = enc_c(sample 2t+h)
                C = ps_comm.tile([64, 2, N], f32, tag="C")
                nc.tensor.matmul(
                    out=C, lhsT=wc_sb, rhs=nbg, start=True, stop=True
                )
                enc = enc_pool.tile([64, 2, N], bf16, tag="enc")
                nc.scalar.activation(
                    out=enc, in_=C, func=AF.Relu, bias=bc_col, scale=1.0
                )
                enc_live[g] = enc

            emit_front(0)
            for g in range(G):
                if g + 1 < G:
                    emit_front(g + 1)
                nb = nb_live.pop(g)
                enc = enc_live.pop(g)

                # attention logits first (exp gates the DVE chain):
                # A[k, (h,t), n] = logit_k(sample 2t+h); each MM-h writes
                # one contiguous PSUM bank of the A tile
                A = ps_att.tile([128, 2, 2, N], f32, tag="A")
                for h in range(2):
                    nc.tensor.matmul(
                        out=A[:, h, :, :], lhsT=wah[h], rhs=enc[:, :, :],
                        start=True, stop=True,
                    )

                # exp of the whole group in one ACT op, PSUM -> SBUF bf16
                e = e_pool.tile([128, 2, 2, N], bf16, tag="e")
                nc.scalar.activation(out=e, in_=A, func=AF.Exp)

                # feature encoder (transposed): F_h[k, t, n] = feat_k(2t+h);
                # when bf != 0 it is added via an accumulating K=1 matmul
                F = []
                for h in range(2):
                    Fh = ps_feat.tile([128, 2, N], f32, tag="F")
                    nc.tensor.matmul(
                        out=Fh, lhsT=wfh[h], rhs=nb[:, :, :],
                        start=True, stop=not with_bias,
                    )
                    if with_bias:
                        nc.tensor.matmul(
                            out=Fh, lhsT=bf_row, rhs=ones512[:, :, :],
                            start=False, stop=True,
                        )
                    F.append(Fh)

                # fused num/den: one custom DVE op per sample
                for h in range(2):
                    for t in range(2):
                        j = 2 * t + h
                        s = 4 * g + j
                        nc.vector._custom_dve(
                            mrs_op,
                            out=scr_all[:, g, j, :],
                            in0=e[:, h, t, :],
                            in1=F[h][:, t, :],
                            s0=float(N - 1),
                            accum_out=acc_blk[:, s : s + 1],
                        )

                # e255 tap on Pool (SBUF->SBUF copy, sample order s=2t+h):
                # the finale solves den = accum - num + e255
                e4 = p255_blk[:, 4 * g : 4 * g + 4].rearrange(
                    "p (t h2) -> p h2 t", h2=2
                )
                nc.gpsimd.tensor_scalar(
                    out=e4, in0=e[:, :, :, N - 1], scalar1=1.0, scalar2=None,
                    op0=ALU.mult,
                )

            # finale: num = scr[..., 255]; den = acc - num + e255;
            # out = relu((num/den) @ Wl + bl).  Two half-pipelines so the
            # first half's matmul/relu/DMA overlaps the second's compute.
            num_v = scr_all[:, :, :, N - 1].rearrange("p g j -> p (g j)")
            den_t = singles.tile([LAT, S], f32)
            rden = singles.tile([LAT, S], f32)
            aggT = singles.tile([LAT, S], bf16)
            out_sb = singles.tile([S, LAT], f32)
            H = S // 2
            for hf in range(2):
                sl = slice(H * hf, H * hf + H)
                nc.vector.tensor_tensor(
                    out=den_t[:, sl], in0=acc_blk[:, sl], in1=num_v[:, sl],
                    op=ALU.subtract,
                )
                nc.vector.tensor_add(
                    out=den_t[:, sl], in0=den_t[:, sl], in1=p255_blk[:, sl]
                )
                nc.vector.reciprocal(out=rden[:, sl], in_=den_t[:, sl])
                nc.vector.tensor_mul(
                    out=aggT[:, sl], in0=num_v[:, sl], in1=rden[:, sl]
                )
                out_ps_w = ps_feat.tile([128, 2, N], f32, tag="F")
                out_ps = out_ps_w.rearrange("p t n -> p (t n)")[0:H, 0:LAT]
                nc.tensor.matmul(
                    out=out_ps, lhsT=aggT[:, sl], rhs=wl_sb,
                    start=True, stop=False,
                )
                nc.tensor.matmul(
                    out=out_ps, lhsT=ones1[:, 0:H], rhs=blr,
                    start=False, stop=True,
                )
                nc.scalar.activation(
                    out=out_sb[sl, :], in_=out_ps, func=AF.Relu
                )
                nc.sync.dma_start(out=out_d[sl, :], in_=out_sb[sl, :])

    nc.finalize()
    return nc


def _host_prep(inputs):
    import ml_dtypes

    bf = np.asarray(inputs["bf"], dtype=np.float32)
    bc = np.asarray(inputs["bc"], dtype=np.float32)
    bl = np.asarray(inputs["bl"], dtype=np.float32)
    Wf = np.asarray(inputs["Wf"], dtype=np.float32)
    Wc = np.asarray(inputs["Wc"], dtype=np.float32)
    Wa = np.asarray(inputs["Wa"], dtype=np.float32)
    Wl = np.asarray(inputs["Wl"], dtype=np.float32)
    Wa2 = Wa[EC : 2 * EC]  # only the enc_comm block survives softmax shift

    nbr = np.asarray(inputs["neighbor_data"], dtype=np.float32)
    # [M, S, N, D] -> [M, S, D, N] -> [M, S*D, N], bf16
    nbrT = (
        np.ascontiguousarray(nbr.reshape(M, S, N, D).transpose(0, 1, 3, 2))
        .reshape(M, S * D, N)
        .astype(ml_dtypes.bfloat16)
    )

    wtsb = np.zeros((128, 964), dtype=ml_dtypes.bfloat16)
    wtsb[0:64, 0:128] = Wf
    wtsb[64:128, 128:256] = Wf
    wtsb[0:64, 256:288] = Wc
    wtsb[64:128, 288:320] = Wc
    wtsb[0:32, 320:448] = Wa2
    wtsb[32:64, 448:576] = Wa2
    wtsb[:, 576:704] = Wl
    wtsb[0, 704:832] = bl
    wtsb[0, 832:960] = bf

    wtsf = np.zeros((128, 2), dtype=np.float32)
    wtsf[:, 0] = bf
    wtsf[0:64, 1] = np.tile(bc, 2)
    wtsb[:, 960:964] = wtsf.view(np.uint16).view(ml_dtypes.bfloat16)

    return [{"nbrT": nbrT[c], "wtsb": wtsb} for c in range(M)]


def kernel(**inputs) -> np.ndarray:
    from concourse.bass_utils import run_bass_kernel_spmd

    with_bias = bool(np.any(np.asarray(inputs["bf"])))
    key = ("nc", with_bias)
    if key not in _CACHE:
        _CACHE[key] = _build_bass(with_bias)
    nc = _CACHE[key]

    in_maps = _host_prep(inputs)
    res = run_bass_kernel_spmd(nc, in_maps, list(range(M)))
    out = np.concatenate(
        [np.asarray(res.results[c]["out"]) for c in range(M)], axis=0
    )
    return out.astype(np.float32)


# revision 5
# speedup vs baseline: 7570.6020x; 1.0005x over previous
"""Trainium2 Bass kernel for AttentiveRelationalModuleUniformObs (v5).

Math (per sample b over N=256 neighbors, D=64, LAT=128, EC=32):
    feat   = relu(nbr @ Wf + bf)            [N, LAT]
    enc    = relu(nbr @ Wc + bc)            [N, EC]
    att    = softmax_N(enc @ Wa2)           [N, LAT]   (shift-invariance:
             self/mean/ba logit terms are constant over N and cancel)
    out[b] = relu((att * feat).sum(N) @ Wl + bl)

Strategy:
  - All matmuls bf16 (full PE rate, halved HBM traffic); layout A
    (LAT on partitions, neighbors on the free axis).
  - ONE custom DVE op per sample (MUL_RELUBIAS_SCANSEL) computes both
    softmax reductions in a single 1x pass over (e, F):
        body   = select(Idx >= 255, scan(+, e), e * relu(F + bf))
        out    = prod prefixes, except out[:, 255] = sum_n e  (= den)
        accum  = sum(body) = (num - prod[255]) + den
    A tiny Pool op recovers prod[255]; the finale solves
        num = accum - den + prod255.
    So DVE does 4x[128,256] ops per group and nothing else.
  - ACT: one exp per group [128,1024] PSUM->SBUF bf16.
  - Pool: enc-relu (tensor_scalar add-bias/max-0) + the prod255 taps.
  - Engine budget per 4-sample group ~ DVE 1.57us / Pool 1.38us /
    ACT 1.04us / PE 1.30us / DMA 0.78us -> DVE-paced ~51us/core.
"""

import numpy as np

B, N, D, LAT, EC = 1024, 256, 64, 128, 32
M = 8           # cores
S = B // M      # samples per core (128)
G = S // 4      # main-loop iterations per core (4 samples each)

_CACHE = {}

_MRS_NAME = "MUL_RELUBIAS_SCANSEL"
_MRS_ROW = 17  # custom-DVE opcode rows 1..16 are taken by production ops


def _register_mrs():
    """Register the custom DVE op at runtime (pre-seeding the compile
    cache in place of a pinned uops sha):
        body  = select(Idx >= s0, scan(add, in0), in0 * relu(in1 + s1))
        accum = sum(body)
    """
    import concourse.dve_ops as dops
    from concourse.dve_spec import (
        AluOp, Spec, Src0, Src1, C0, C1, Zero, Idx, relu, scan, select, lower,
    )
    from concourse.dve_spec import _has_src1
    from concourse.dve_uop import DveOpSpec
    from operator import add

    if _MRS_NAME in dops._SUB_OPCODE_FOR_NAME:
        return next(op for op in dops.OPS if op.name == _MRS_NAME)

    def _ref(in0, in1, c0, c1, c2):
        in0f = in0.astype(np.float32).reshape(in0.shape[0], -1)
        in1f = in1.astype(np.float32).reshape(in0.shape[0], -1)
        prod = in0f * np.maximum(np.nan_to_num(in1f), 0.0)
        cump = np.cumsum(prod, axis=-1)
        idx = np.arange(in0f.shape[-1])[None, :]
        body = np.where(idx >= c0, cump, in0f).astype(np.float32)
        return (
            body.reshape(in0.shape),
            body.sum(axis=-1, keepdims=True),
        )

    body = select(Idx >= C0, scan(AluOp.ADD, Src0 * relu(Src1)), Src0)
    spec = Spec(body=body, accum=add, accum_init=Zero, reference=_ref)
    op = dops.DveOp(_MRS_NAME, spec, subdim=False, uops_sha={})
    dops.OPS.append(op)
    dops.CUSTOM_DVE_SPECS[_MRS_NAME] = spec
    dops._SUB_OPCODE_FOR_NAME[_MRS_NAME] = _MRS_ROW
    for ver in ("v3", "v4"):
        compiled = DveOpSpec(
            name=_MRS_NAME,
            opcode=_MRS_ROW,
            uops=lower(spec, ver=ver),
            rd1_en=_has_src1(spec),
        )
        dops._COMPILE_CACHE[(_MRS_NAME, ver)] = compiled
    return op


def _build_bass(with_bias: bool, bench_reps: int = 1):
    import concourse.bacc as bacc
    import concourse.tile as tile
    from concourse import mybir

    mrs_op = _register_mrs()

    f32 = mybir.dt.float32
    bf16 = mybir.dt.bfloat16
    AF = mybir.ActivationFunctionType
    ALU = mybir.AluOpType

    nc = bacc.Bacc("TRN2", target_bir_lowering=False)

    # DRAM I/O.  nbrT: [S*D, N] bf16, host pre-transposed (D on partitions).
    nbrT_d = nc.dram_tensor("nbrT", [S * D, N], bf16, kind="ExternalInput")
    # bf16 weights packed into one tensor (one DMA):
    # cols [0:128) WfH0 | [128:256) WfH1 | [256:320) WcS2 blockdiag
    #      | [320:448) WaH0 (rows 0:32) | [448:576) WaH1 (rows 32:64)
    #      | [576:704) Wl | [704:832) bl (row 0) | [832:960) bf (row 0)
    #      | [960:964) f32 bias cols (bitcast): bf | bc-tiled-2x
    wtsb_d = nc.dram_tensor("wtsb", [128, 964], bf16, kind="ExternalInput")
    out_d = nc.dram_tensor("out", [S, LAT], f32, kind="ExternalOutput")

    with tile.TileContext(nc) as tc:
        from contextlib import ExitStack

        with ExitStack() as ctx:
            singles = ctx.enter_context(tc.tile_pool(name="singles", bufs=1))
            nbr_pool = ctx.enter_context(tc.tile_pool(name="nbr", bufs=5))
            enc_pool = ctx.enter_context(tc.tile_pool(name="enc", bufs=4))
            e_pool = ctx.enter_context(tc.tile_pool(name="e", bufs=3))
            # PSUM: att 2x[128,4,256] (4 banks) + feat 3x[128,2,256] (3) +
            # comm 1x[64,2,256] (1) = 8 banks; the finale reuses the feat tag.
            ps_att = ctx.enter_context(
                tc.tile_pool(name="ps_att", bufs=2, space="PSUM")
            )
            ps_feat = ctx.enter_context(
                tc.tile_pool(name="ps_feat", bufs=3, space="PSUM")
            )
            ps_comm = ctx.enter_context(
                tc.tile_pool(name="ps_comm", bufs=1, space="PSUM")
            )

            wtsb = singles.tile([128, 964], bf16)
            nc.sync.dma_start(out=wtsb, in_=wtsb_d[:, :])
            wtsf = wtsb[:, 960:964].bitcast(f32)
            wfh = [wtsb[:, 0:128], wtsb[:, 128:256]]
            wc_sb = wtsb[:, 256:320]
            wah = [wtsb[0:64, 320:448], wtsb[0:64, 448:576]]
            wl_sb = wtsb[:, 576:704]
            blr = wtsb[0:1, 704:832]
            bf_row = wtsb[0:1, 832:960]
            bc_col = wtsf[0:64, 1:2]
            ones1 = singles.tile([1, LAT], bf16)
            nc.vector.memset(ones1, 1.0)
            if with_bias:
                ones512 = singles.tile([1, 2, N], bf16)
                nc.vector.memset(ones512, 1.0)
            # warm the Exp activation table while the weight DMAs are in
            # flight so LoadActFuncSet isn't on the first group's chain
            warm = singles.tile([1, 1], f32)
            nc.vector.memset(warm, 0.0)
            warm2 = singles.tile([1, 1], f32)
            nc.scalar.activation(out=warm2, in_=warm, func=AF.Exp)

            # custom-op outputs: prod-prefix scratch (col 255 = den),
            # accum column (= num - prod255 + den), prod255 taps
            scr_all = singles.tile([128, G, 4, N], bf16)
            acc_blk = singles.tile([LAT, S], f32)
            p255_blk = singles.tile([LAT, S], f32)

            nbrT_ap = nbrT_d[:, :]

            if bench_reps > 1:
                ctx.enter_context(tc.For_i(0, bench_reps))

            # Software-pipelined: comm+enc for group g+1 are emitted before
            # the back half of group g so Pool's enc-relu stays off the
            # enc->att->exp critical cycle.
            nb_live = {}
            enc_live = {}

            def emit_front(g):
                # 4 samples: partition p = 64*h + d, free index t; sample
                # s = 4g + 2t + h.  1KB-contiguous rows in DRAM per (t,p).
                r0 = 4 * g * D
                nbg = nbr_pool.tile([128, 2, N], bf16, tag="nbr")
                nc.sync.dma_start(
                    out=nbg,
                    in_=nbrT_ap[r0 : r0 + 256, :].rearrange(
                        "(t p) n -> p t n", p=128
                    ),
                )
                nb_live[g] = nbg

                # communication encoder via block-diagonal WcS2:
                # C[32h+c, t, n] = enc_c(sample 2t+h)
                C = ps_comm.tile([64, 2, N], f32, tag="C")
                nc.tensor.matmul(
                    out=C, lhsT=wc_sb, rhs=nbg, start=True, stop=True
                )
                enc = enc_pool.tile([64, 2, N], bf16, tag="enc")
                nc.scalar.activation(
                    out=enc, in_=C, func=AF.Relu, bias=bc_col, scale=1.0
                )
                enc_live[g] = enc

            emit_front(0)
            for g in range(G):
                if g + 1 < G:
                    emit_front(g + 1)
                nb = nb_live.pop(g)
                enc = enc_live.pop(g)

                # attention logits first (exp gates the DVE chain):
                # A[k, (h,t), n] = logit_k(sample 2t+h); each MM-h writes
                # one contiguous PSUM bank of the A tile
                A = ps_att.tile([128, 2, 2, N], f32, tag="A")
                for h in range(2):
                    nc.tensor.matmul(
                        out=A[:, h, :, :], lhsT=wah[h], rhs=enc[:, :, :],
                        start=True, stop=True,
                    )

                # exp of the whole group in one ACT op, PSUM -> SBUF bf16
                e = e_pool.tile([128, 2, 2, N], bf16, tag="e")
                nc.scalar.activation(out=e, in_=A, func=AF.Exp)

                # feature encoder (transposed): F_h[k, t, n] = feat_k(2t+h);
                # when bf != 0 it is added via an accumulating K=1 matmul
                F = []
                for h in range(2):
                    Fh = ps_feat.tile([128, 2, N], f32, tag="F")
                    nc.tensor.matmul(
                        out=Fh, lhsT=wfh[h], rhs=nb[:, :, :],
                        start=True, stop=not with_bias,
                    )
                    if with_bias:
                        nc.tensor.matmul(
                            out=Fh, lhsT=bf_row, rhs=ones512[:, :, :],
                            start=False, stop=True,
                        )
                    F.append(Fh)

                # fused num/den: one custom DVE op per sample
                for h in range(2):
                    for t in range(2):
                        j = 2 * t + h
                        s = 4 * g + j
                        nc.vector._custom_dve(
                            mrs_op,
                            out=scr_all[:, g, j, :],
                            in0=e[:, h, t, :],
                            in1=F[h][:, t, :],
                            s0=float(N - 1),
                            accum_out=acc_blk[:, s : s + 1],
                        )

                # e255 tap on Pool (SBUF->SBUF copy, sample order s=2t+h):
                # the finale solves den = accum - num + e255
                e4 = p255_blk[:, 4 * g : 4 * g + 4].rearrange(
                    "p (t h2) -> p h2 t", h2=2
                )
                nc.gpsimd.tensor_scalar(
                    out=e4, in0=e[:, :, :, N - 1], scalar1=1.0, scalar2=None,
                    op0=ALU.mult,
                )

            # finale: num = scr[..., 255]; den = acc - num + e255;
            # out = relu((num/den) @ Wl + bl).  Two half-pipelines so the
            # first half's matmul/relu/DMA overlaps the second's compute.
            num_v = scr_all[:, :, :, N - 1].rearrange("p g j -> p (g j)")
            den_t = singles.tile([LAT, S], f32)
            rden = singles.tile([LAT, S], f32)
            aggT = singles.tile([LAT, S], bf16)
            out_sb = singles.tile([S, LAT], f32)
            H = S // 2
            for hf in range(2):
                sl = slice(H * hf, H * hf + H)
                nc.vector.tensor_tensor(
                    out=den_t[:, sl], in0=acc_blk[:, sl], in1=num_v[:, sl],
                    op=ALU.subtract,
                )
                nc.vector.tensor_add(
                    out=den_t[:, sl], in0=den_t[:, sl], in1=p255_blk[:, sl]
                )
                nc.vector.reciprocal(out=rden[:, sl], in_=den_t[:, sl])
                nc.vector.tensor_mul(
                    out=aggT[:, sl], in0=num_v[:, sl], in1=rden[:, sl]
                )
                out_ps_w = ps_feat.tile([128, 2, N], f32, tag="F")
                out_ps = out_ps_w.rearrange("p t n -> p (t n)")[0:H, 0:LAT]
                nc.tensor.matmul(
                    out=out_ps, lhsT=aggT[:, sl], rhs=wl_sb,
                    start=True, stop=False,
                )
                nc.tensor.matmul(
                    out=out_ps, lhsT=ones1[:, 0:H], rhs=blr,
                    start=False, stop=True,
                )
                nc.scalar.activation(
                    out=out_sb[sl, :], in_=out_ps, func=AF.Relu
                )
                nc.sync.dma_start(out=out_d[sl, :], in_=out_sb[sl, :])

    nc.finalize()
    return nc


def _host_prep(inputs):
    import ml_dtypes

    bf = np.asarray(inputs["bf"], dtype=np.float32)
    bc = np.asarray(inputs["bc"], dtype=np.float32)
    bl = np.asarray(inputs["bl"], dtype=np.float32)
    Wf = np.asarray(inputs["Wf"], dtype=np.float32)
    Wc = np.asarray(inputs["Wc"], dtype=np.float32)
    Wa = np.asarray(inputs["Wa"], dtype=np.float32)
    Wl = np.asarray(inputs["Wl"], dtype=np.float32)
    Wa2 = Wa[EC : 2 * EC]  # only the enc_comm block survives softmax shift

    nbr = np.asarray(inputs["neighbor_data"], dtype=np.float32)
    # [M, S, N, D] -> [M, S, D, N] -> [M, S*D, N], bf16
    nbrT = (
        np.ascontiguousarray(nbr.reshape(M, S, N, D).transpose(0, 1, 3, 2))
        .reshape(M, S * D, N)
        .astype(ml_dtypes.bfloat16)
    )

    wtsb = np.zeros((128, 964), dtype=ml_dtypes.bfloat16)
    wtsb[0:64, 0:128] = Wf
    wtsb[64:128, 128:256] = Wf
    wtsb[0:64, 256:288] = Wc
    wtsb[64:128, 288:320] = Wc
    wtsb[0:32, 320:448] = Wa2
    wtsb[32:64, 448:576] = Wa2
    wtsb[:, 576:704] = Wl
    wtsb[0, 704:832] = bl
    wtsb[0, 832:960] = bf

    wtsf = np.zeros((128, 2), dtype=np.float32)
    wtsf[:, 0] = bf
    wtsf[0:64, 1] = np.tile(bc, 2)
    wtsb[:, 960:964] = wtsf.view(np.uint16).view(ml_dtypes.bfloat16)

    return [{"nbrT": nbrT[c], "wtsb": wtsb} for c in range(M)]


def kernel(**inputs) -> np.ndarray:
    from concourse.bass_utils import run_bass_kernel_spmd

    with_bias = bool(np.any(np.asarray(inputs["bf"])))
    key = ("nc", with_bias)
    if key not in _CACHE:
        _CACHE[key] = _build_bass(with_bias)
    nc = _CACHE[key]

    in_maps = _host_prep(inputs)
    res = run_bass_kernel_spmd(nc, in_maps, list(range(M)))
    out = np.concatenate(
        [np.asarray(res.results[c]["out"]) for c in range(M)], axis=0
    )
    return out.astype(np.float32)
